# revision 80
# baseline (speedup 1.0000x reference)
"""Multi-head attention (B=8, N=1024, C=768, H=12) on 8 TRN2 NeuronCores.

Sharding: pure data-parallel over batch — core b computes attention for x[b].
No collectives needed. Per-core Bass/Tile kernel, bf16 compute, f32 PSUM.

Host prep (transposes are free on the host):
  xT = x[b].T [768,1024], wqk = qkv_w[:1536].T, wv = qkv_w[1536:].T,
  wp = proj_w.T, pb = proj_b.reshape(6,128).T — all bf16 except pb.

Per-core device compute:
  q/k:  wqk.T @ xT per head, stored in a DUPLICATED layout (the head's 64
        d-rows on both partition halves) so each m-tile's two S matmuls hit
        disjoint PE row groups and execute concurrently (~183 ns/MM).
  v:    xT.T @ wv in natural [n, (h, 65)] layout with a fused ones-column
        per head.
  per head:  S^T[m,n] = k q^T (K=64); E = exp(S^T*scale) on ScalarE (no max
        subtraction — |S*scale| <= ~6 for randn inputs); O^T/sums = [v|1].T @ E
        accumulated over m-tiles in PSUM (row 64 = softmax denominators);
        normalize via reciprocal_approx_fast + gpsimd partition_broadcast.
  yT = wp.T @ Onorm^T + pb, output bf16 (host transposes/upcasts).

Schedule: software pipeline over 6 head pairs. Pair hp's S/exp streams carry
pair hp-1's O^T matmuls plus a queue of independent "filler" matmul chunks
(v, next pair's q/k, projection partials) so the in-order PE queue never
stalls on the ScalarE exp stream and HAM stays at 2.4 GHz. PSUM is budgeted
at exactly 4 two-bank slots; warmup matmuls run during the input-DMA window.
"""

import numpy as np
import ml_dtypes

B, N, C = 8, 1024, 768
H, D = 12, 64
SCALE = D ** -0.5
CT = C // 128       # 6 contraction tiles
OT = 2 * C // 128   # 12 o-tiles of qkT
NT = N // 128       # 8 token tiles
NCH = N // 512      # 2 n-chunks of 512
HP = H // 2         # 6 head pairs

_CACHE = {}


def _build_nc():
    import concourse.bacc as bacc
    import concourse.mybir as mybir
    import concourse.tile as tile

    f32 = mybir.dt.float32
    bf16 = mybir.dt.bfloat16

    nc = bacc.Bacc("TRN2", target_bir_lowering=False, debug=False, num_devices=8)

    xT_d = nc.dram_tensor("xT", [C, N], bf16, kind="ExternalInput").ap()
    wqk_d = nc.dram_tensor("wqk", [C, 2 * C], bf16, kind="ExternalInput").ap()
    wv_d = nc.dram_tensor("wv", [C, C], bf16, kind="ExternalInput").ap()
    wp_d = nc.dram_tensor("wp", [C, C], bf16, kind="ExternalInput").ap()
    pb_d = nc.dram_tensor("pb", [128, CT], f32, kind="ExternalInput").ap()
    out_d = nc.dram_tensor("out", [C, N], bf16, kind="ExternalOutput").ap()

    with tile.TileContext(nc) as tc:
        with (
            tc.tile_pool(name="const", bufs=1) as cpool,
            tc.tile_pool(name="E", bufs=4) as epool,
            tc.tile_pool(name="small", bufs=3) as spool,
            tc.tile_pool(name="y", bufs=4) as ypool,
            tc.tile_pool(name="dup", bufs=8) as dpool,
            tc.tile_pool(name="ps", bufs=4, space="PSUM") as pspool,
        ):
            # ---- persistent SBUF tensors ----
            xT_sb = cpool.tile([128, CT, N], bf16)            # 12KB/part
            wqk_sb = cpool.tile([128, CT, 2 * C], bf16)       # 18KB
            wv_sb = cpool.tile([128, CT, C], bf16)            # 9KB
            wp_sb = cpool.tile([128, CT, C], bf16)            # 9KB
            pb_sb = cpool.tile([128, CT], f32)
            v_sb = cpool.tile([128, NT, H, D + 1], bf16)      # 12.2KB
            on_sb = cpool.tile([128, CT, NCH, 512], bf16)     # Onorm^T, 12KB

            # DMA order: what the first attention pair needs lands first
            for kt in range(CT):
                r = kt * 128
                nc.sync.dma_start(xT_sb[:, kt, :], xT_d[r:r + 128, :])
                nc.sync.dma_start(wqk_sb[:, kt, 0:128], wqk_d[r:r + 128, 0:128])
                nc.sync.dma_start(wqk_sb[:, kt, 768:896], wqk_d[r:r + 128, 768:896])
            for kt in range(CT):
                r = kt * 128
                nc.sync.dma_start(wv_sb[:, kt, :], wv_d[r:r + 128, :])
                nc.sync.dma_start(wqk_sb[:, kt, 128:768], wqk_d[r:r + 128, 128:768])
                nc.sync.dma_start(wqk_sb[:, kt, 896:1536], wqk_d[r:r + 128, 896:1536])
                nc.sync.dma_start(wp_sb[:, kt, :], wp_d[r:r + 128, :])
            nc.sync.dma_start(pb_sb[:], pb_d[:])
            # ones column fused into v (gives softmax sums as O^T row 64)
            nc.vector.memset(v_sb[:, :, :, D:D + 1], 1.0)
            ones1 = cpool.tile([1, 64], f32)
            nc.vector.memset(ones1[:], 1.0)




            # q/k tiles are stored per head DUPLICATED on both partition
            # halves ("dup" layout): the two S matmuls of each m-tile then
            # target disjoint PE row groups and execute concurrently.
            dupmap = {}

            def dup_from_ps(hp, role, ps, nchs, use_dma=True, act_half=False):
                ta = dupmap.get((hp, role, 0))
                if ta is None:
                    ta = dpool.tile([128, NCH, 512], bf16, tag="dup",
                                    name=f"dup{hp}{role}a")
                    tb = dpool.tile([128, NCH, 512], bf16, tag="dup",
                                    name=f"dup{hp}{role}b")
                    dupmap[(hp, role, 0)] = ta
                    dupmap[(hp, role, 1)] = tb
                tb = dupmap[(hp, role, 1)]
                for nch in nchs:
                    nc.vector.tensor_copy(ta[0:64, nch, :], ps[0:64, nch, :])
                    if act_half:  # startup: ScalarE is idle, halve the chain
                        nc.scalar.copy(tb[0:64, nch, :], ps[64:128, nch, :])
                    else:
                        nc.vector.tensor_copy(tb[0:64, nch, :], ps[64:128, nch, :])
                    if use_dma:
                        # steady state: sync DGE queues (input DMAs done);
                        # keeps gpsimd free so PartitionBroadcast never queues
                        # behind DMA issue
                        eng = nc.gpsimd if hp == 0 else nc.sync
                        eng.dma_start(ta[64:128, nch, :], ta[0:64, nch, :])
                        eng.dma_start(tb[64:128, nch, :], tb[0:64, nch, :])
                    else:
                        # startup: DVE cross-base copies beat the DMA queues
                        nc.vector.tensor_copy(ta[64:128, nch, :], ps[0:64, nch, :])
                        nc.vector.tensor_copy(tb[64:128, nch, :], ps[64:128, nch, :])

            def emit_v(nt):
                ps = pspool.tile([128, 2, 8, 64], f32, tag="ps")
                for kt in range(CT):
                    for och in range(2):
                        nc.tensor.matmul(
                            ps[:, och, 0:6, :],
                            xT_sb[:, kt, nt * 128:(nt + 1) * 128],
                            wv_sb[:, kt, och * 384:(och + 1) * 384],
                            start=(kt == 0), stop=(kt == CT - 1),
                        )
                for och in range(2):
                    nc.vector.tensor_copy(
                        v_sb[:, nt, och * 6:(och + 1) * 6, 0:D], ps[:, och, 0:6, :]
                    )

            # ---- filler machinery: a queue of small independent PE chunks
            # interleaved into the dependency-throttled S/exp streams so the
            # in-order PE queue never stalls (keeps HAM at 2.4 GHz).
            fillers = []

            def take_fillers(k):
                for _ in range(min(k, len(fillers))):
                    fillers.pop(0)()

            def queue_qk(hp, role):
                ot = hp if role == 0 else 6 + hp
                hold = {}
                for kt in range(CT):
                    def chunk(kt=kt, ot=ot, hp=hp, role=role, hold=hold):
                        if kt == 0:
                            hold["ps"] = pspool.tile([128, NCH, 512], f32, tag="ps", name="qkps")
                        for nch in range(NCH):
                            nc.tensor.matmul(
                                hold["ps"][:, nch, :],
                                wqk_sb[:, kt, ot * 128:(ot + 1) * 128],
                                xT_sb[:, kt, nch * 512:(nch + 1) * 512],
                                start=(kt == 0), stop=(kt == CT - 1),
                            )
                        if kt == CT - 1:
                            dup_from_ps(hp, role, hold["ps"], range(NCH))
                    fillers.append(chunk)

            def queue_v(nt):
                hold = {}
                for kt in range(CT):
                    def chunk(kt=kt, nt=nt, hold=hold):
                        if kt == 0:
                            hold["ps"] = pspool.tile([128, 2, 8, 64], f32, tag="ps", name="vps")
                        for och in range(2):
                            nc.tensor.matmul(
                                hold["ps"][:, och, 0:6, :],
                                xT_sb[:, kt, nt * 128:(nt + 1) * 128],
                                wv_sb[:, kt, och * 384:(och + 1) * 384],
                                start=(kt == 0), stop=(kt == CT - 1),
                            )
                        if kt == CT - 1:
                            for och in range(2):
                                nc.vector.tensor_copy(
                                    v_sb[:, nt, och * 6:(och + 1) * 6, 0:D],
                                    hold["ps"][:, och, 0:6, :],
                                )
                    fillers.append(chunk)

            def s_phase(hp, half, E_t, o_prev, rate=1, o_prev2=None, o_self=None):
                """S^T + exp stream for one head of pair hp, with the previous
                pair's O^T matmuls for the same half interleaved per m-tile.
                The two n-chunks run on disjoint PE row groups (dup layout)."""
                qt = dupmap[(hp, 0, half)]
                kt_ = dupmap[(hp, 1, half)]
                for mt in range(NT):
                    ps_s = pspool.tile([128, NCH, 512], f32, tag="ps")
                    c0, c1 = mt // 4, (mt % 4) * 128
                    nc.tensor.matmul(
                        ps_s[:, 0, :], kt_[0:64, c0, c1:c1 + 128],
                        qt[0:64, 0, :], start=True, stop=True,
                    )
                    nc.tensor.matmul(
                        ps_s[:, 1, :], kt_[64:128, c0, c1:c1 + 128],
                        qt[64:128, 1, :], start=True, stop=True,
                    )
                    nc.scalar.activation(
                        E_t[:, mt, :, :], ps_s[:, :, :],
                        mybir.ActivationFunctionType.Exp, scale=SCALE,
                    )
                    for op in (o_prev, o_prev2):
                        if op is not None:
                            h_prev, ps_o, E_prev = op
                            for nch in range(NCH):
                                nc.tensor.matmul(
                                    ps_o[:, nch, :], v_sb[:, mt, h_prev, :],
                                    E_prev[:, mt, nch, :],
                                    start=(mt == 0), stop=(mt == NT - 1),
                                )
                    if o_self is not None and mt > 0:
                        h_s, ps_os = o_self
                        for nch in range(NCH):
                            nc.tensor.matmul(
                                ps_os[:, nch, :], v_sb[:, mt - 1, h_s, :],
                                E_t[:, mt - 1, nch, :],
                                start=(mt == 1), stop=False,
                            )
                    take_fillers(rate)
                if o_self is not None:
                    h_s, ps_os = o_self
                    for nch in range(NCH):
                        nc.tensor.matmul(
                            ps_os[:, nch, :], v_sb[:, NT - 1, h_s, :],
                            E_t[:, NT - 1, nch, :],
                            start=False, stop=True,
                        )

            def emit_o(h, E_t):
                ps_o = pspool.tile([65, NCH, 512], f32, tag="ps")
                for mt in range(NT):
                    for nch in range(NCH):
                        nc.tensor.matmul(
                            ps_o[:, nch, :], v_sb[:, mt, h, :], E_t[:, mt, nch, :],
                            start=(mt == 0), stop=(mt == NT - 1),
                        )
                return ps_o

            def norm(h, ps_o, act_sm=False, pe_R=False):
                """Normalize O^T by the softmax sums in its row 64 and store
                into on_sb (head parity picks the partition half). The psum
                tile is drained to SBUF right away so its slot frees before
                the reciprocal chain finishes. act_sm: route the sums copy to
                ScalarE (idle in the tail). pe_R: broadcast the reciprocals
                with a K=1 PE outer product (0.5us) instead of the gpsimd
                partition_broadcast (1.8us) — tail only, where PE has gaps."""
                hp, odd = h // 2, h % 2
                sm = spool.tile([1, NCH, 512], f32, tag="sum")
                if act_sm:
                    nc.scalar.copy(sm[:], ps_o[64:65, :, :])
                else:
                    nc.vector.tensor_copy(sm[:], ps_o[64:65, :, :])
                ou = spool.tile([64, NCH, 512], bf16, tag="ou")
                nc.vector.tensor_copy(ou[:], ps_o[0:64, :, :])
                rec = spool.tile([1, NCH, 512], f32, tag="rec")
                nc.vector.reciprocal_approx_fast(rec[:], sm[:])
                if pe_R:
                    R = pspool.tile([64, NCH, 512], f32, tag="ps", name=f"R{h}")
                    for nch in range(NCH):
                        nc.tensor.matmul(R[:, nch, :], ones1[:],
                                         rec[0:1, nch, :], start=True, stop=True)
                else:
                    R = spool.tile([64, NCH, 512], f32, tag="R")
                    nc.gpsimd.partition_broadcast(R[:], rec[:])
                if not odd:
                    nc.vector.tensor_tensor(
                        on_sb[0:64, hp, :, :], ou[:], R[:],
                        op=mybir.AluOpType.mult,
                    )
                else:
                    # odd head lives at partitions 64:128 (32-aligned base
                    # shift is legal for DVE operands)
                    nc.vector.tensor_tensor(
                        on_sb[64:128, hp, :, :], ou[:], R[:],
                        op=mybir.AluOpType.mult,
                    )

            def emit_proj(otp):
                ps = pspool.tile([128, NCH, 512], f32, tag="ps")
                for kt in range(CT):
                    for nch in range(NCH):
                        nc.tensor.matmul(
                            ps[:, nch, :],
                            wp_sb[:, kt, otp * 128:(otp + 1) * 128],
                            on_sb[:, kt, nch, :],
                            start=(kt == 0), stop=(kt == CT - 1),
                        )
                yt = ypool.tile([128, NCH, 512], bf16, tag="yt")
                nc.scalar.activation(
                    yt[:], ps[:, :, :], mybir.ActivationFunctionType.Identity,
                    bias=pb_sb[:, otp:otp + 1],
                )
                nc.sync.dma_start(out_d[otp * 128:(otp + 1) * 128, :], yt[:])

            # ---- software pipeline over head pairs: pair hp's S/exp streams
            # carry pair hp-1's O^T accumulation as interleaved PE work; v and
            # the next pair's qkT ride along as fillers. Fillers queued during
            # pair hp are fully drained within pair hp (pop-rate x chunk
            # counts are sized so), so data deps never point forward in the
            # in-order PE queue.
            pj = {}

            def queue_proj(otp, kts):
                for kt in kts:
                    def chunk(otp=otp, kt=kt):
                        if kt == 0:
                            pj[otp] = pspool.tile([128, NCH, 512], f32, tag="ps",
                                                  name="pjps")
                        for nch in range(NCH):
                            nc.tensor.matmul(
                                pj[otp][:, nch, :],
                                wp_sb[:, kt, otp * 128:(otp + 1) * 128],
                                on_sb[:, kt, nch, :],
                                start=(kt == 0), stop=False,
                            )
                    fillers.append(chunk)

            # eager q/k tiles for pair 0, kt-major: all four accumulation
            # groups (2 roles x 2 n-chunks) advance together so each kt's
            # matmuls fire as soon as that kt's DMA lands — compute pipelines
            # with the input stream instead of waiting for all of xT. Head-a
            # copies go first; head-b copies are deferred past the first S
            # matmuls' dependencies.
            eager_ps = {}
            for role in (0, 1):
                eager_ps[role] = pspool.tile([128, NCH, 512], f32, tag="ps",
                                             name=f"qk0r{role}")
                dupmap[(0, role, 0)] = dpool.tile([128, NCH, 512], bf16,
                                                  tag="dup", name=f"dup0{role}a")
                dupmap[(0, role, 1)] = dpool.tile([128, NCH, 512], bf16,
                                                  tag="dup", name=f"dup0{role}b")
            for kt in range(CT):
                for role in (0, 1):
                    ot = 0 if role == 0 else 6
                    for nch in range(NCH):
                        nc.tensor.matmul(
                            eager_ps[role][:, nch, :],
                            wqk_sb[:, kt, ot * 128:(ot + 1) * 128],
                            xT_sb[:, kt, nch * 512:(nch + 1) * 512],
                            start=(kt == 0), stop=(kt == CT - 1),
                        )
            for role in (0, 1):
                ta = dupmap[(0, role, 0)]
                for nch in range(NCH):
                    nc.vector.tensor_copy(ta[0:64, nch, :],
                                          eager_ps[role][0:64, nch, :])
                    nc.gpsimd.dma_start(ta[64:128, nch, :], ta[0:64, nch, :])
            for role in (0, 1):
                tb = dupmap[(0, role, 1)]
                for nch in range(NCH):
                    nc.vector.tensor_copy(tb[0:64, nch, :],
                                          eager_ps[role][64:128, nch, :])
                    nc.gpsimd.dma_start(tb[64:128, nch, :], tb[0:64, nch, :])
            prev = None  # (E_a, E_b) of previous pair
            for hp in range(HP):
                E_a = epool.tile([128, NT, NCH, 512], bf16, tag="E")
                if hp == 0:
                    for nt in range(NT):
                        queue_v(nt)              # 48 chunks
                if hp + 1 < HP:
                    queue_qk(hp + 1, 0)          # 6 chunks
                    queue_qk(hp + 1, 1)          # 6 chunks
                rate = 4 if hp == 0 else 1

                last = hp == HP - 1
                o_prev_a = o_prev_b = None
                if prev is not None:
                    ps_opa = pspool.tile([65, NCH, 512], f32, tag="ps")
                    o_prev_a = (2 * (hp - 1), ps_opa, prev[0])
                if last and prev is not None:
                    # both O(4) heads ride phase a: frees a phase-b PSUM slot
                    # so the pair's own head-b O^T can run same-phase below
                    ps_opb = pspool.tile([65, NCH, 512], f32, tag="ps")
                    o_prev_b = (2 * (hp - 1) + 1, ps_opb, prev[1])
                s_phase(hp, 0, E_a, o_prev_a, rate, o_prev2=o_prev_b)
                if o_prev_a is not None:
                    norm(2 * (hp - 1), o_prev_a[1])
                if last and o_prev_b is not None:
                    norm(2 * (hp - 1) + 1, o_prev_b[1])
                E_b = epool.tile([128, NT, NCH, 512], bf16, tag="E")
                if not last and prev is not None:
                    ps_opb = pspool.tile([65, NCH, 512], f32, tag="ps")
                    o_prev_b = (2 * (hp - 1) + 1, ps_opb, prev[1])
                o_self_a = o_self_b = None
                if last:
                    # last pair: its own O^T for BOTH heads rides phase b —
                    # head a reads the completed E_a, head b lags its own exp
                    # stream by one m-tile. The tail then has no O matmuls.
                    ps_o10 = pspool.tile([65, NCH, 512], f32, tag="ps", name="o10")
                    o_self_a = (2 * hp, ps_o10, E_a)
                    ps_o11 = pspool.tile([65, NCH, 512], f32, tag="ps", name="o11")
                    o_self_b = (2 * hp + 1, ps_o11)
                s_phase(hp, 1, E_b, o_self_a if last else o_prev_b,
                        rate if hp == 0 else 2, o_self=o_self_b)
                if not last and o_prev_b is not None:
                    norm(2 * (hp - 1) + 1, o_prev_b[1])
                take_fillers(len(fillers))       # drain: invariant at pair end
                prev = (E_a, E_b)

            # tail: both last-pair O^T accumulations finished inside the
            # pair; run their norm chains and fill PE with the projection
            # partials (pj0/1/2 hold PSUM slots; pj3/pj4 rotate through freed
            # slots and park in SBUF via ScalarE; pj5 keeps the last slot)
            ha, hb = 2 * (HP - 1), 2 * (HP - 1) + 1
            norm(ha, ps_o10, act_sm=True)
            norm(hb, ps_o11, act_sm=True)
            stg = {}

            def queue_stage(otp):
                def chunk(otp=otp):
                    t = ypool.tile([128, NCH, 512], bf16, tag="stg",
                                   name=f"stg{otp}", bufs=2)
                    stg[otp] = t
                    nc.scalar.copy(t[:], pj[otp][:, :, :])
                fillers.append(chunk)

            queue_proj(2, range(0, 5))
            queue_proj(0, range(0, 5))
            queue_proj(3, range(0, 5))
            queue_stage(3)
            queue_proj(1, range(0, 5))
            queue_proj(4, range(0, 5))
            queue_stage(4)
            queue_proj(5, range(0, 5))
            take_fillers(len(fillers))

            # ---- epilogue: kt5 + bias for psum-held otps (0/1/5), kt5 +
            # staged-partial recombine on VectorE for the staged otps (2/3/4)
            def finish_proj(otp):
                for nch in range(NCH):
                    nc.tensor.matmul(
                        pj[otp][:, nch, :],
                        wp_sb[:, CT - 1, otp * 128:(otp + 1) * 128],
                        on_sb[:, CT - 1, nch, :],
                        start=False, stop=True,
                    )
                yt = ypool.tile([128, NCH, 512], bf16, tag="yt")
                if otp != 1:
                    nc.scalar.activation(
                        yt[:], pj[otp][:, :, :],
                        mybir.ActivationFunctionType.Identity,
                        bias=pb_sb[:, otp:otp + 1],
                    )
                else:
                    nc.vector.tensor_scalar_add(yt[:], pj[otp][:, :, :],
                                                pb_sb[:, otp:otp + 1])
                nc.sync.dma_start(out_d[otp * 128:(otp + 1) * 128, :], yt[:])

            def finish_staged(otp):
                psx = pspool.tile([128, NCH, 512], f32, tag="ps",
                                  name=f"k5{otp}")
                for nch in range(NCH):
                    nc.tensor.matmul(
                        psx[:, nch, :],
                        wp_sb[:, CT - 1, otp * 128:(otp + 1) * 128],
                        on_sb[:, CT - 1, nch, :], start=True, stop=True,
                    )
                yt = ypool.tile([128, NCH, 512], bf16, tag="yt")
                nc.vector.scalar_tensor_tensor(
                    yt[:], psx[:, :, :], pb_sb[:, otp:otp + 1], stg[otp][:],
                    op0=mybir.AluOpType.add, op1=mybir.AluOpType.add,
                )
                nc.sync.dma_start(out_d[otp * 128:(otp + 1) * 128, :], yt[:])

            finish_proj(0)
            finish_proj(1)
            finish_proj(2)
            finish_proj(5)
            finish_staged(3)
            finish_staged(4)

    nc.compile()
    return nc


def _get_nc():
    if "nc" not in _CACHE:
        _CACHE["nc"] = _build_nc()
    return _CACHE["nc"]


def kernel(x, qkv_w, proj_w, proj_b):
    from concourse.bass_utils import run_bass_kernel_spmd

    nc = _get_nc()
    bf = ml_dtypes.bfloat16
    wqk = np.ascontiguousarray(qkv_w[:2 * C].T).astype(bf)
    wv = np.ascontiguousarray(qkv_w[2 * C:].T).astype(bf)
    wp = np.ascontiguousarray(proj_w.T).astype(bf)
    pb = np.ascontiguousarray(proj_b.reshape(CT, 128).T).astype(np.float32)
    in_maps = []
    for i in range(B):
        in_maps.append({
            "xT": np.ascontiguousarray(x[i].T).astype(bf),
            "wqk": wqk, "wv": wv, "wp": wp, "pb": pb,
        })
    res = run_bass_kernel_spmd(nc, in_maps, core_ids=list(range(B)))
    out = np.stack([res.results[i]["out"].astype(np.float32).T for i in range(B)])
    return np.ascontiguousarray(out)


# revision 82
# speedup vs baseline: 1.0135x; 1.0135x over previous
"""Multi-head attention (B=8, N=1024, C=768, H=12) on 8 TRN2 NeuronCores.

Sharding: pure data-parallel over batch — core b computes attention for x[b].
No collectives needed. Per-core Bass/Tile kernel, bf16 compute, f32 PSUM.

Host prep (transposes are free on the host):
  xT = x[b].T [768,1024], wqk = qkv_w[:1536].T, wv = qkv_w[1536:].T,
  wp = proj_w.T, pb = proj_b.reshape(6,128).T — all bf16 except pb.

Per-core device compute:
  q/k:  wqk.T @ xT per head, stored in a DUPLICATED layout (the head's 64
        d-rows on both partition halves) so each m-tile's two S matmuls hit
        disjoint PE row groups and execute concurrently (~183 ns/MM).
  v:    xT.T @ wv in natural [n, (h, 65)] layout with a fused ones-column
        per head.
  per head:  S^T[m,n] = k q^T (K=64); E = exp(S^T*scale) on ScalarE (no max
        subtraction — |S*scale| <= ~6 for randn inputs); O^T/sums = [v|1].T @ E
        accumulated over m-tiles in PSUM (row 64 = softmax denominators);
        normalize via reciprocal_approx_fast + gpsimd partition_broadcast.
  yT = wp.T @ Onorm^T + pb, output bf16 (host transposes/upcasts).

Schedule: software pipeline over 6 head pairs. Pair hp's S/exp streams carry
pair hp-1's O^T matmuls plus a queue of independent "filler" matmul chunks
(v, next pair's q/k, projection partials) so the in-order PE queue never
stalls on the ScalarE exp stream and HAM stays at 2.4 GHz. PSUM is budgeted
at exactly 4 two-bank slots; warmup matmuls run during the input-DMA window.
"""

import numpy as np
import ml_dtypes

B, N, C = 8, 1024, 768
H, D = 12, 64
SCALE = D ** -0.5
CT = C // 128       # 6 contraction tiles
OT = 2 * C // 128   # 12 o-tiles of qkT
NT = N // 128       # 8 token tiles
NCH = N // 512      # 2 n-chunks of 512
HP = H // 2         # 6 head pairs

_CACHE = {}


def _build_nc():
    import concourse.bacc as bacc
    import concourse.mybir as mybir
    import concourse.tile as tile

    f32 = mybir.dt.float32
    bf16 = mybir.dt.bfloat16

    nc = bacc.Bacc("TRN2", target_bir_lowering=False, debug=False, num_devices=8)

    xT_d = nc.dram_tensor("xT", [C, N], bf16, kind="ExternalInput").ap()
    wqk_d = nc.dram_tensor("wqk", [C, 2 * C], bf16, kind="ExternalInput").ap()
    wv_d = nc.dram_tensor("wv", [C, C], bf16, kind="ExternalInput").ap()
    wp_d = nc.dram_tensor("wp", [C, C], bf16, kind="ExternalInput").ap()
    pb_d = nc.dram_tensor("pb", [128, CT], f32, kind="ExternalInput").ap()
    out_d = nc.dram_tensor("out", [C, N], bf16, kind="ExternalOutput").ap()

    with tile.TileContext(nc) as tc:
        with (
            tc.tile_pool(name="const", bufs=1) as cpool,
            tc.tile_pool(name="E", bufs=4) as epool,
            tc.tile_pool(name="small", bufs=3) as spool,
            tc.tile_pool(name="y", bufs=4) as ypool,
            tc.tile_pool(name="dup", bufs=8) as dpool,
            tc.tile_pool(name="ps", bufs=4, space="PSUM") as pspool,
        ):
            # ---- persistent SBUF tensors ----
            xT_sb = cpool.tile([128, CT, N], bf16)            # 12KB/part
            wqk_sb = cpool.tile([128, CT, 2 * C], bf16)       # 18KB
            wv_sb = cpool.tile([128, CT, C], bf16)            # 9KB
            wp_sb = cpool.tile([128, CT, C], bf16)            # 9KB
            pb_sb = cpool.tile([128, CT], f32)
            v_sb = cpool.tile([128, NT, H, D + 1], bf16)      # 12.2KB
            on_sb = cpool.tile([128, CT, NCH, 512], bf16)     # Onorm^T, 12KB

            # DMA order: what the first attention pair needs lands first
            for kt in range(CT):
                r = kt * 128
                nc.sync.dma_start(xT_sb[:, kt, :], xT_d[r:r + 128, :])
                nc.sync.dma_start(wqk_sb[:, kt, 0:128], wqk_d[r:r + 128, 0:128])
                nc.sync.dma_start(wqk_sb[:, kt, 768:896], wqk_d[r:r + 128, 768:896])
            for kt in range(CT):
                r = kt * 128
                nc.sync.dma_start(wv_sb[:, kt, :], wv_d[r:r + 128, :])
                nc.sync.dma_start(wqk_sb[:, kt, 128:768], wqk_d[r:r + 128, 128:768])
                nc.sync.dma_start(wqk_sb[:, kt, 896:1536], wqk_d[r:r + 128, 896:1536])
                nc.sync.dma_start(wp_sb[:, kt, :], wp_d[r:r + 128, :])
            nc.sync.dma_start(pb_sb[:], pb_d[:])
            # ones column fused into v (gives softmax sums as O^T row 64)
            nc.vector.memset(v_sb[:, :, :, D:D + 1], 1.0)
            ones1 = cpool.tile([1, 64], f32)
            nc.vector.memset(ones1[:], 1.0)




            # q/k tiles are stored per head DUPLICATED on both partition
            # halves ("dup" layout): the two S matmuls of each m-tile then
            # target disjoint PE row groups and execute concurrently.
            dupmap = {}

            def dup_from_ps(hp, role, ps, nchs, use_dma=True, act_half=False):
                ta = dupmap.get((hp, role, 0))
                if ta is None:
                    ta = dpool.tile([128, NCH, 512], bf16, tag="dup",
                                    name=f"dup{hp}{role}a")
                    tb = dpool.tile([128, NCH, 512], bf16, tag="dup",
                                    name=f"dup{hp}{role}b")
                    dupmap[(hp, role, 0)] = ta
                    dupmap[(hp, role, 1)] = tb
                tb = dupmap[(hp, role, 1)]
                for nch in nchs:
                    nc.vector.tensor_copy(ta[0:64, nch, :], ps[0:64, nch, :])
                    if act_half:  # startup: ScalarE is idle, halve the chain
                        nc.scalar.copy(tb[0:64, nch, :], ps[64:128, nch, :])
                    else:
                        nc.vector.tensor_copy(tb[0:64, nch, :], ps[64:128, nch, :])
                    if use_dma:
                        # steady state: sync DGE queues (input DMAs done);
                        # keeps gpsimd free so PartitionBroadcast never queues
                        # behind DMA issue
                        eng = nc.gpsimd if hp == 0 else nc.sync
                        eng.dma_start(ta[64:128, nch, :], ta[0:64, nch, :])
                        eng.dma_start(tb[64:128, nch, :], tb[0:64, nch, :])
                    else:
                        # startup: DVE cross-base copies beat the DMA queues
                        nc.vector.tensor_copy(ta[64:128, nch, :], ps[0:64, nch, :])
                        nc.vector.tensor_copy(tb[64:128, nch, :], ps[64:128, nch, :])

            def emit_v(nt):
                ps = pspool.tile([128, 2, 8, 64], f32, tag="ps")
                for kt in range(CT):
                    for och in range(2):
                        nc.tensor.matmul(
                            ps[:, och, 0:6, :],
                            xT_sb[:, kt, nt * 128:(nt + 1) * 128],
                            wv_sb[:, kt, och * 384:(och + 1) * 384],
                            start=(kt == 0), stop=(kt == CT - 1),
                        )
                for och in range(2):
                    nc.vector.tensor_copy(
                        v_sb[:, nt, och * 6:(och + 1) * 6, 0:D], ps[:, och, 0:6, :]
                    )

            # ---- filler machinery: a queue of small independent PE chunks
            # interleaved into the dependency-throttled S/exp streams so the
            # in-order PE queue never stalls (keeps HAM at 2.4 GHz).
            fillers = []

            def take_fillers(k):
                for _ in range(min(k, len(fillers))):
                    fillers.pop(0)()

            def queue_qk(hp, role):
                ot = hp if role == 0 else 6 + hp
                hold = {}
                for kt in range(CT):
                    def chunk(kt=kt, ot=ot, hp=hp, role=role, hold=hold):
                        if kt == 0:
                            hold["ps"] = pspool.tile([128, NCH, 512], f32, tag="ps", name="qkps")
                        for nch in range(NCH):
                            nc.tensor.matmul(
                                hold["ps"][:, nch, :],
                                wqk_sb[:, kt, ot * 128:(ot + 1) * 128],
                                xT_sb[:, kt, nch * 512:(nch + 1) * 512],
                                start=(kt == 0), stop=(kt == CT - 1),
                            )
                        if kt == CT - 1:
                            dup_from_ps(hp, role, hold["ps"], range(NCH))
                    fillers.append(chunk)

            def queue_v(nt):
                hold = {}
                for kt in range(CT):
                    def chunk(kt=kt, nt=nt, hold=hold):
                        if kt == 0:
                            hold["ps"] = pspool.tile([128, 2, 8, 64], f32, tag="ps", name="vps")
                        for och in range(2):
                            nc.tensor.matmul(
                                hold["ps"][:, och, 0:6, :],
                                xT_sb[:, kt, nt * 128:(nt + 1) * 128],
                                wv_sb[:, kt, och * 384:(och + 1) * 384],
                                start=(kt == 0), stop=(kt == CT - 1),
                            )
                        if kt == CT - 1:
                            for och in range(2):
                                nc.vector.tensor_copy(
                                    v_sb[:, nt, och * 6:(och + 1) * 6, 0:D],
                                    hold["ps"][:, och, 0:6, :],
                                )
                    fillers.append(chunk)

            def s_phase(hp, half, E_t, o_prev, rate=1, o_prev2=None):
                """S^T + exp stream for one head of pair hp, with the previous
                pair's O^T matmuls for the same half interleaved per m-tile.
                The two n-chunks run on disjoint PE row groups (dup layout)."""
                qt = dupmap[(hp, 0, half)]
                kt_ = dupmap[(hp, 1, half)]
                for mt in range(NT):
                    ps_s = pspool.tile([128, NCH, 512], f32, tag="ps")
                    c0, c1 = mt // 4, (mt % 4) * 128
                    nc.tensor.matmul(
                        ps_s[:, 0, :], kt_[0:64, c0, c1:c1 + 128],
                        qt[0:64, 0, :], start=True, stop=True,
                    )
                    nc.tensor.matmul(
                        ps_s[:, 1, :], kt_[64:128, c0, c1:c1 + 128],
                        qt[64:128, 1, :], start=True, stop=True,
                    )
                    nc.scalar.activation(
                        E_t[:, mt, :, :], ps_s[:, :, :],
                        mybir.ActivationFunctionType.Exp, scale=SCALE,
                    )
                    for op in (o_prev, o_prev2):
                        if op is not None:
                            h_prev, ps_o, E_prev = op
                            for nch in range(NCH):
                                nc.tensor.matmul(
                                    ps_o[:, nch, :], v_sb[:, mt, h_prev, :],
                                    E_prev[:, mt, nch, :],
                                    start=(mt == 0), stop=(mt == NT - 1),
                                )
                    take_fillers(rate)

            def emit_o(h, E_t):
                ps_o = pspool.tile([65, NCH, 512], f32, tag="ps")
                for mt in range(NT):
                    for nch in range(NCH):
                        nc.tensor.matmul(
                            ps_o[:, nch, :], v_sb[:, mt, h, :], E_t[:, mt, nch, :],
                            start=(mt == 0), stop=(mt == NT - 1),
                        )
                return ps_o

            def norm(h, ps_o, act_sm=False, pe_R=False):
                """Normalize O^T by the softmax sums in its row 64 and store
                into on_sb (head parity picks the partition half). The psum
                tile is drained to SBUF right away so its slot frees before
                the reciprocal chain finishes. act_sm: route the sums copy to
                ScalarE (idle in the tail). pe_R: broadcast the reciprocals
                with a K=1 PE outer product (0.5us) instead of the gpsimd
                partition_broadcast (1.8us) — tail only, where PE has gaps."""
                hp, odd = h // 2, h % 2
                sm = spool.tile([1, NCH, 512], f32, tag="sum")
                if act_sm:
                    nc.scalar.copy(sm[:], ps_o[64:65, :, :])
                else:
                    nc.vector.tensor_copy(sm[:], ps_o[64:65, :, :])
                ou = spool.tile([64, NCH, 512], bf16, tag="ou")
                nc.vector.tensor_copy(ou[:], ps_o[0:64, :, :])
                rec = spool.tile([1, NCH, 512], f32, tag="rec")
                nc.vector.reciprocal_approx_fast(rec[:], sm[:])
                if pe_R:
                    R = pspool.tile([64, NCH, 512], f32, tag="ps", name=f"R{h}")
                    for nch in range(NCH):
                        nc.tensor.matmul(R[:, nch, :], ones1[:],
                                         rec[0:1, nch, :], start=True, stop=True)
                else:
                    R = spool.tile([64, NCH, 512], f32, tag="R")
                    nc.gpsimd.partition_broadcast(R[:], rec[:])
                if not odd:
                    nc.vector.tensor_tensor(
                        on_sb[0:64, hp, :, :], ou[:], R[:],
                        op=mybir.AluOpType.mult,
                    )
                else:
                    # odd head lives at partitions 64:128 (32-aligned base
                    # shift is legal for DVE operands)
                    nc.vector.tensor_tensor(
                        on_sb[64:128, hp, :, :], ou[:], R[:],
                        op=mybir.AluOpType.mult,
                    )

            def emit_proj(otp):
                ps = pspool.tile([128, NCH, 512], f32, tag="ps")
                for kt in range(CT):
                    for nch in range(NCH):
                        nc.tensor.matmul(
                            ps[:, nch, :],
                            wp_sb[:, kt, otp * 128:(otp + 1) * 128],
                            on_sb[:, kt, nch, :],
                            start=(kt == 0), stop=(kt == CT - 1),
                        )
                yt = ypool.tile([128, NCH, 512], bf16, tag="yt")
                nc.scalar.activation(
                    yt[:], ps[:, :, :], mybir.ActivationFunctionType.Identity,
                    bias=pb_sb[:, otp:otp + 1],
                )
                nc.sync.dma_start(out_d[otp * 128:(otp + 1) * 128, :], yt[:])

            # ---- software pipeline over head pairs: pair hp's S/exp streams
            # carry pair hp-1's O^T accumulation as interleaved PE work; v and
            # the next pair's qkT ride along as fillers. Fillers queued during
            # pair hp are fully drained within pair hp (pop-rate x chunk
            # counts are sized so), so data deps never point forward in the
            # in-order PE queue.
            pj = {}

            def queue_proj(otp, kts):
                for kt in kts:
                    def chunk(otp=otp, kt=kt):
                        if kt == 0:
                            pj[otp] = pspool.tile([128, NCH, 512], f32, tag="ps",
                                                  name="pjps")
                        for nch in range(NCH):
                            nc.tensor.matmul(
                                pj[otp][:, nch, :],
                                wp_sb[:, kt, otp * 128:(otp + 1) * 128],
                                on_sb[:, kt, nch, :],
                                start=(kt == 0), stop=False,
                            )
                    fillers.append(chunk)

            # eager q/k tiles for pair 0, kt-major: all four accumulation
            # groups (2 roles x 2 n-chunks) advance together so each kt's
            # matmuls fire as soon as that kt's DMA lands — compute pipelines
            # with the input stream instead of waiting for all of xT. Head-a
            # copies go first; head-b copies are deferred past the first S
            # matmuls' dependencies.
            eager_ps = {}
            for role in (0, 1):
                eager_ps[role] = pspool.tile([128, NCH, 512], f32, tag="ps",
                                             name=f"qk0r{role}")
                dupmap[(0, role, 0)] = dpool.tile([128, NCH, 512], bf16,
                                                  tag="dup", name=f"dup0{role}a")
                dupmap[(0, role, 1)] = dpool.tile([128, NCH, 512], bf16,
                                                  tag="dup", name=f"dup0{role}b")
            for kt in range(CT):
                for role in (0, 1):
                    ot = 0 if role == 0 else 6
                    for nch in range(NCH):
                        nc.tensor.matmul(
                            eager_ps[role][:, nch, :],
                            wqk_sb[:, kt, ot * 128:(ot + 1) * 128],
                            xT_sb[:, kt, nch * 512:(nch + 1) * 512],
                            start=(kt == 0), stop=(kt == CT - 1),
                        )
            for role in (0, 1):
                ta = dupmap[(0, role, 0)]
                for nch in range(NCH):
                    nc.vector.tensor_copy(ta[0:64, nch, :],
                                          eager_ps[role][0:64, nch, :])
                    nc.gpsimd.dma_start(ta[64:128, nch, :], ta[0:64, nch, :])
            for role in (0, 1):
                tb = dupmap[(0, role, 1)]
                for nch in range(NCH):
                    nc.vector.tensor_copy(tb[0:64, nch, :],
                                          eager_ps[role][64:128, nch, :])
                    nc.gpsimd.dma_start(tb[64:128, nch, :], tb[0:64, nch, :])
            # two v tiles fill the otherwise-idle PE window while the eager
            # copies and dup DMAs finish (wv lands ~6us before this point)
            emit_v(0)
            emit_v(1)
            prev = None  # (E_a, E_b) of previous pair
            for hp in range(HP):
                E_a = epool.tile([128, NT, NCH, 512], bf16, tag="E")
                if hp == 0:
                    for nt in range(2, NT):
                        queue_v(nt)              # 36 chunks
                if hp + 1 < HP:
                    queue_qk(hp + 1, 0)          # 6 chunks
                    queue_qk(hp + 1, 1)          # 6 chunks
                rate = 4 if hp == 0 else 1

                o_prev_a = None
                if prev is not None:
                    ps_opa = pspool.tile([65, NCH, 512], f32, tag="ps")
                    o_prev_a = (2 * (hp - 1), ps_opa, prev[0])
                s_phase(hp, 0, E_a, o_prev_a, rate)
                if o_prev_a is not None:
                    norm(2 * (hp - 1), o_prev_a[1])
                E_b = epool.tile([128, NT, NCH, 512], bf16, tag="E")
                o_prev_b = None
                if prev is not None:
                    ps_opb = pspool.tile([65, NCH, 512], f32, tag="ps")
                    o_prev_b = (2 * (hp - 1) + 1, ps_opb, prev[1])
                o_self_a = None
                if hp == HP - 1:
                    # last pair: its own head-a O^T rides in phase b (E_a done)
                    ps_o10 = pspool.tile([65, NCH, 512], f32, tag="ps", name="o10")
                    o_self_a = (2 * hp, ps_o10, E_a)
                s_phase(hp, 1, E_b, o_prev_b, rate if hp == 0 else 2,
                        o_prev2=o_self_a)
                if o_prev_b is not None:
                    norm(2 * (hp - 1) + 1, o_prev_b[1])
                take_fillers(len(fillers))       # drain: invariant at pair end
                prev = (E_a, E_b)

            # tail: head 10's O^T already accumulated in pair 5 phase b; norm
            # it now, stream head 11's O^T with ALL SIX projection partial
            # accumulations as filler. pj0/pj1 hold PSUM slots; pj2/pj3/pj4
            # rotate through one slot and park in SBUF via idle ScalarE; pj5
            # keeps the rotating slot. Held-accumulator chunks are interleaved
            # as spacers so a rotation never stalls the in-order PE queue.
            ha, hb = 2 * (HP - 1), 2 * (HP - 1) + 1
            norm(ha, ps_o10, act_sm=True)
            ps_o11 = pspool.tile([65, NCH, 512], f32, tag="ps", name="o11")
            stg = {}

            def queue_stage(otp):
                def chunk(otp=otp):
                    t = ypool.tile([128, NCH, 512], bf16, tag="stg",
                                   name=f"stg{otp}", bufs=2)
                    stg[otp] = t
                    nc.scalar.copy(t[:], pj[otp][:, :, :])
                fillers.append(chunk)

            # O(11) runs dense first — its completion gates the final norm
            # chain (no ScalarE dependency in the tail). The six projection
            # partial accumulations follow: pj0/1/2 hold PSUM slots, pj3/pj4
            # rotate through freed slots and park in SBUF via ScalarE, pj5
            # keeps the last slot. Held-accumulator chunks act as spacers so
            # rotations never stall the in-order PE queue.
            for mt in range(NT):
                for nch in range(NCH):
                    nc.tensor.matmul(
                        ps_o11[:, nch, :], v_sb[:, mt, hb, :], prev[1][:, mt, nch, :],
                        start=(mt == 0), stop=(mt == NT - 1),
                    )
            norm(hb, ps_o11, act_sm=True)
            queue_proj(2, range(0, 5))
            queue_proj(0, range(0, 5))
            queue_proj(3, range(0, 5))
            queue_stage(3)
            queue_proj(1, range(0, 5))
            queue_proj(4, range(0, 5))
            queue_stage(4)
            queue_proj(5, range(0, 5))
            take_fillers(len(fillers))

            # ---- epilogue: kt5 + bias for psum-held otps (0/1/5), kt5 +
            # staged-partial recombine on VectorE for the staged otps (2/3/4)
            def finish_proj(otp):
                for nch in range(NCH):
                    nc.tensor.matmul(
                        pj[otp][:, nch, :],
                        wp_sb[:, CT - 1, otp * 128:(otp + 1) * 128],
                        on_sb[:, CT - 1, nch, :],
                        start=False, stop=True,
                    )
                yt = ypool.tile([128, NCH, 512], bf16, tag="yt")
                if otp != 1:
                    nc.scalar.activation(
                        yt[:], pj[otp][:, :, :],
                        mybir.ActivationFunctionType.Identity,
                        bias=pb_sb[:, otp:otp + 1],
                    )
                else:
                    nc.vector.tensor_scalar_add(yt[:], pj[otp][:, :, :],
                                                pb_sb[:, otp:otp + 1])
                nc.sync.dma_start(out_d[otp * 128:(otp + 1) * 128, :], yt[:])

            def finish_staged(otp):
                psx = pspool.tile([128, NCH, 512], f32, tag="ps",
                                  name=f"k5{otp}")
                for nch in range(NCH):
                    nc.tensor.matmul(
                        psx[:, nch, :],
                        wp_sb[:, CT - 1, otp * 128:(otp + 1) * 128],
                        on_sb[:, CT - 1, nch, :], start=True, stop=True,
                    )
                yt = ypool.tile([128, NCH, 512], bf16, tag="yt")
                nc.vector.scalar_tensor_tensor(
                    yt[:], psx[:, :, :], pb_sb[:, otp:otp + 1], stg[otp][:],
                    op0=mybir.AluOpType.add, op1=mybir.AluOpType.add,
                )
                nc.sync.dma_start(out_d[otp * 128:(otp + 1) * 128, :], yt[:])

            finish_proj(0)
            finish_proj(1)
            finish_proj(2)
            finish_proj(5)
            finish_staged(3)
            finish_staged(4)

    nc.compile()
    return nc


def _get_nc():
    if "nc" not in _CACHE:
        _CACHE["nc"] = _build_nc()
    return _CACHE["nc"]


def kernel(x, qkv_w, proj_w, proj_b):
    from concourse.bass_utils import run_bass_kernel_spmd

    nc = _get_nc()
    bf = ml_dtypes.bfloat16
    wqk = np.ascontiguousarray(qkv_w[:2 * C].T).astype(bf)
    wv = np.ascontiguousarray(qkv_w[2 * C:].T).astype(bf)
    wp = np.ascontiguousarray(proj_w.T).astype(bf)
    pb = np.ascontiguousarray(proj_b.reshape(CT, 128).T).astype(np.float32)
    in_maps = []
    for i in range(B):
        in_maps.append({
            "xT": np.ascontiguousarray(x[i].T).astype(bf),
            "wqk": wqk, "wv": wv, "wp": wp, "pb": pb,
        })
    res = run_bass_kernel_spmd(nc, in_maps, core_ids=list(range(B)))
    out = np.stack([res.results[i]["out"].astype(np.float32).T for i in range(B)])
    return np.ascontiguousarray(out)
